# revision 18
# baseline (speedup 1.0000x reference)
"""Trainium2 Bass kernel for CheemsNonWoAttn (GQA attention block, no Wo).

Sharding: 8 cores = batch(2) x kv-head-pair(4). Each core handles one batch
element and 2 of the 8 kv heads (GQA: Q and K are repeated identically across
the 4 groups, so only 8 unique softmax matrices exist; V uses all 32 heads).

Per-core device program:
  Qt/Kt = Wq^T X^T, Wk^T X^T    (d on partitions; N=512 matmuls, K accum=16)
  RoPE on DVE (partition-shifted multiplies, sign folded into sin table)
  V = X Wv                      (natural [s, 512] layout)
  scores^T[k, q] = Kt^T Qt      (per head, K=64, row-tiled across array halves)
  E = exp(scores) on ACT        (no max subtraction: |scores| <~ 6, safe)
  causal mask: multiply 128x128 diagonal blocks by triangular mask (DVE)
  out = E^T V with a ones-column in V producing the softmax denominator
  out /= denom (DVE reciprocal + tensor_scalar)
"""
import os
import sys
import types
from contextlib import ExitStack

for _p in ("/opt/trn_rl_repo", "/root/.axon_site/_ro/trn_rl_repo"):
    if os.path.isdir(_p) and _p not in sys.path:
        sys.path.append(_p)

import numpy as np
import ml_dtypes

import concourse.bass as bass
import concourse.tile as tile
from concourse import mybir
from concourse.bass_utils import run_bass_kernel_spmd
from concourse.vector_clock import ScopedClock

# ---------------------------------------------------------------------------
# Patch 1: walrus rejects Drain instructions with >1 sync wait (CTRL ops have
# a single wait slot). Split the TileContext exit drain's waits across extra
# SP nops, one wait each.
def _patched_drain_and_barrier(self, tick_clock, wait_clock):
    nc = self.nc
    drain_bi = nc.sync.drain()
    wait_clock.add_sem_waits(drain_bi.ins, ScopedClock({None: tick_clock.global_clock}))
    inst = drain_bi.ins
    si = inst.sync_info
    if si is not None and si.on_wait is not None and len(si.on_wait) > 1:
        waits = list(si.on_wait)
        inst.sync_info = mybir.SyncInfo(
            on_wait=waits[:1],
            on_update=list(si.on_update) if si.on_update else [],
        )
        for w in waits[1:]:
            nbi = nc.sync.nop()
            nbi.ins.sync_info = mybir.SyncInfo(on_wait=[w], on_update=[])
    nc.all_engine_barrier()
    assert self.sems is not None
    popped = nc._tile_sem_poison_stack.pop()
    assert popped is self._sem_poison
    nc.clear_and_free_semaphores(list(self.sems.allocated().values()))
    nc.all_engine_barrier()


tile.TileContext._drain_and_barrier = _patched_drain_and_barrier


def _legalize_waits(nc):
    """This walrus build accepts at most one sync-wait per instruction.
    Split any instruction carrying N>1 waits into N-1 preceding same-engine
    nops (engines are in-order, so semantics are preserved)."""
    uid = 0
    for f in nc.m.functions:
        for blk in f.blocks:
            insts = list(blk.instructions)
            out, changed = [], False
            for inst in insts:
                si = getattr(inst, "sync_info", None)
                if si is not None and si.on_wait is not None and len(si.on_wait) > 1:
                    waits = list(si.on_wait)
                    for w in waits[:-1]:
                        uid += 1
                        out.append(mybir.InstNoOp(
                            name=f"{inst.name}_lw{uid}",
                            engine=inst.engine,
                            sync_info=mybir.SyncInfo(on_wait=[w], on_update=[]),
                            bass_nofuse=True,
                        ))
                    inst.sync_info = mybir.SyncInfo(
                        on_wait=waits[-1:],
                        on_update=list(si.on_update) if si.on_update else [],
                    )
                    changed = True
                out.append(inst)
            if changed:
                blk.instructions = out


def _dedup_ldweights(nc):
    """Clear the self-load on matmuls whose stationary operand AP is identical
    to the immediately preceding matmul in the final PE stream (walrus runs
    with ldw-opt disabled, so it reloads weights for every matmul otherwise)."""
    if os.environ.get("CHEEMS_NO_LDW_DEDUP"):
        return
    for f in nc.m.functions:
        for blk in f.blocks:
            prev_key = None
            for inst in blk.instructions:
                if not isinstance(inst, mybir.InstMatmult):
                    if isinstance(inst, mybir.InstLdweights):
                        prev_key = None
                    continue
                key = (repr(inst.ins[1]), inst.perf_mode, inst.is_transpose,
                       repr(inst.tile_position))
                if prev_key is not None and key == prev_key:
                    inst.ldweights = False
                prev_key = key


# Patch 2 (optional, for tracing): recreate the antenv.axon_hooks shim so
# run_bass_kernel_spmd(trace=True) can capture NTFF profiles under axon.
def _install_ntff_hook():
    try:
        if "antenv.axon_hooks" in sys.modules:
            return
        import antenv
        from trn_agent_boot.trn_boot import _ntff_profile_via_ctypes

        hook = _ntff_profile_via_ctypes("/opt/axon/libaxon_pjrt.so")
        mod = types.ModuleType("antenv.axon_hooks")
        mod._hook = hook
        mod.get_axon_ntff_profile_hook = lambda: mod._hook

        def _set(h):
            mod._hook = h

        mod.set_axon_ntff_profile_hook = _set
        sys.modules["antenv.axon_hooks"] = mod
        antenv.axon_hooks = mod
    except Exception:
        pass


# ---------------------------------------------------------------------------
B, S, HID = 2, 2048, 2048
NH, G = 32, 4
HD = 64          # head dim
HKV = 8          # kv heads
THETA = 10000.0
P = 128          # partitions
NKT = HID // P   # 16 k-tiles over the contraction dim
NST = S // P     # 16 s-tiles
NCH = 4          # s-chunks of 512
CH = 512
DV = 512         # v columns per core (8 heads x 64)
VROW = 528       # v tile row: [256 v | 1 one | 7 pad] x 2 heads
VOFF = 264

F32 = mybir.dt.float32
BF16 = mybir.dt.bfloat16

_CACHE = {}
LAST_RESULTS = None


def _build():
    nc = bass.Bass("TRN2")
    d_xt = nc.declare_dram_parameter("xt", [HID, S], BF16, isOutput=False)
    # weights pre-transposed on host to partition-major [P, kt*cols] so the
    # DMA is a dense linear stream (strided gathers were ~4x slower)
    d_wq = nc.declare_dram_parameter("wq", [P, NKT * P], BF16, isOutput=False)
    d_wk = nc.declare_dram_parameter("wk", [P, NKT * P], BF16, isOutput=False)
    d_wv = nc.declare_dram_parameter("wv", [P, NKT * DV], BF16, isOutput=False)
    d_cos = nc.declare_dram_parameter("cost", [P, S], BF16, isOutput=False)
    d_sin = nc.declare_dram_parameter("sint", [P, S], BF16, isOutput=False)
    d_tri = nc.declare_dram_parameter("tri", [P, P], BF16, isOutput=False)
    d_out = nc.declare_dram_parameter("out", [S, DV], F32, isOutput=True)

    with tile.TileContext(nc) as tc, ExitStack() as ctx:
        pers = ctx.enter_context(tc.tile_pool(name="pers", bufs=1))
        epool = ctx.enter_context(tc.tile_pool(name="epool", bufs=56))
        work = ctx.enter_context(tc.tile_pool(name="work", bufs=2))
        outp = ctx.enter_context(tc.tile_pool(name="outp", bufs=3))
        psum = ctx.enter_context(tc.tile_pool(name="psum", bufs=8, space="PSUM"))

        def ps_tile(name):
            return psum.tile([P, CH], F32, tag="ps", bufs=8, name=name)

        # --- persistent tiles; DMA order tuned so PE never starves.
        # Enqueue in parallel across 4 engine queues (sync/vector/scalar/
        # gpsimd) so descriptor writes (~1us each) don't serialize the
        # stream start. Per-queue FIFO order still delivers wq/wk first,
        # then the xt tiles (QK proj is PE-paced behind them), then wv
        # (needed when V proj starts right after QK), then rope tables.
        wq_sb = pers.tile([P, NKT, P], BF16, tag="wq")
        wk_sb = pers.tile([P, NKT, P], BF16, tag="wk")
        wv_sb = pers.tile([P, NKT, DV], BF16, tag="wv")
        cos_sb = pers.tile([P, S], BF16, tag="cos")
        sin_sb = pers.tile([P, S], BF16, tag="sin")
        tri_sb = pers.tile([P, P], BF16, tag="tri")
        xt = [pers.tile([P, S], BF16, tag=f"xt{k}", name=f"xt{k}") for k in range(NKT)]
        # xt streams in k-order on the sync HW queue at full bandwidth (QK
        # proj consumes tiles serially); weights/tables ride the scalar HW
        # queue in parallel. gpsimd DMA is software-DGE (slow) - avoid.
        for k in range(8):
            nc.sync.dma_start(out=xt[k][:], in_=d_xt[bass.ts(k, P), :])
        nc.sync.dma_start(out=cos_sb[:], in_=d_cos[:])
        nc.sync.dma_start(out=sin_sb[:], in_=d_sin[:])
        for k in range(8, NKT):
            nc.sync.dma_start(out=xt[k][:], in_=d_xt[bass.ts(k, P), :])
        nc.scalar.dma_start(out=wq_sb[:], in_=d_wq[:].rearrange("p (kt m) -> p kt m", m=P))
        nc.scalar.dma_start(out=wk_sb[:], in_=d_wk[:].rearrange("p (kt m) -> p kt m", m=P))
        wv_r = d_wv[:].rearrange("p (kt n) -> p kt n", n=DV)
        nc.sync.dma_start(out=wv_sb[:, 0:8, :], in_=wv_r[:, 0:8, :])
        nc.sync.dma_start(out=wv_sb[:, 8:NKT, :], in_=wv_r[:, 8:NKT, :])
        nc.sync.dma_start(out=tri_sb[:], in_=d_tri[:])

        qt = pers.tile([P, S], BF16, tag="qt")
        kt = pers.tile([P, S], BF16, tag="kt")
        v_sb = [pers.tile([P, VROW], BF16, tag=f"v{t}", name=f"v{t}") for t in range(NST)]

        # --- phase 1: Q and K projections interleaved per xt tile (8 MMs per
        # DMA arrival keeps PE duty high while the xt stream lands).
        pq = [ps_tile(f"pq{c}") for c in range(NCH)]
        pk = [ps_tile(f"pk{c}") for c in range(NCH)]
        for k in range(NKT):
            for c in range(NCH):
                nc.tensor.matmul(pq[c][:], lhsT=wq_sb[:, k, :], rhs=xt[k][:, bass.ts(c, CH)],
                                 start=(k == 0), stop=(k == NKT - 1), skip_group_check=True)
            for c in range(NCH):
                nc.tensor.matmul(pk[c][:], lhsT=wk_sb[:, k, :], rhs=xt[k][:, bass.ts(c, CH)],
                                 start=(k == 0), stop=(k == NKT - 1), skip_group_check=True)

        def rope(src_ps, dst, c, eng):
            # stage PSUM->bf16 on ACT, then all-bf16 rope ops (2x DVE rate,
            # and gpsimd-eligible since no PSUM reads)
            cs = bass.ts(c, CH)
            qb = work.tile([P, CH], BF16, tag="qb", name="qb")
            nc.scalar.copy(qb[:], src_ps[:])
            t1 = work.tile([P, CH], BF16, tag="t1", name="t1")
            t2 = work.tile([P, CH], BF16, tag="t2", name="t2")
            eng.tensor_mul(out=t1[:], in0=qb[:], in1=cos_sb[:, cs])
            for blk in range(4):
                lo, hi = blk * 32, (blk + 1) * 32
                swap_lo = (blk ^ 1) * 32
                # sin table is block-swapped on host so both SBUF inputs
                # share a base partition (BIR verifier NCC_IBIR297)
                eng.tensor_mul(out=t2[lo:hi, :], in0=qb[swap_lo:swap_lo + 32, :],
                               in1=sin_sb[swap_lo:swap_lo + 32, cs])
            eng.tensor_add(out=dst[:, cs], in0=t1[:], in1=t2[:])

        # k-rope on gpsimd, q-rope on DVE: they run concurrently, and
        # scores(c) needs kt/qt of chunk<=c first
        for c in range(NCH):
            rope(pk[c], kt, c, nc.gpsimd)
            rope(pq[c], qt, c, nc.vector)

        # --- attention work generators (pumped between V-proj s-tiles) ---
        e_tiles = {}

        def gen_scores(c):
            cs0 = c * CH
            nk = 4 * c + 4
            tiles = [[None] * nk for _ in range(2)]
            e_tiles[c] = tiles
            for t in range(nk):
                m = t - 4 * c
                off = max(m, 0) * P
                w = CH - off
                for h in range(2):
                    ps_s = ps_tile("ps_s")
                    nc.tensor.matmul(
                        ps_s[:, 0:w],
                        lhsT=kt[h * HD:(h + 1) * HD, bass.ts(t, P)],
                        rhs=qt[h * HD:(h + 1) * HD, bass.ds(cs0 + off, w)],
                        start=True, stop=True, skip_group_check=True)
                    e = epool.tile([P, CH], BF16, tag="e", name=f"e{h}_{t}")
                    nc.scalar.activation(e[:, bass.ds(off, w)], ps_s[:, 0:w],
                                         mybir.ActivationFunctionType.Exp)
                    if m >= 0:
                        nc.vector.tensor_mul(out=e[:, bass.ts(m, P)],
                                             in0=e[:, bass.ts(m, P)], in1=tri_sb[:])
                    tiles[h][t] = e
                yield 2  # 2 MMs emitted

        def gen_av(c):
            tiles = e_tiles.pop(c)
            for m in range(4):
                q_idx = 4 * c + m
                out_stage = outp.tile([P, DV], F32, tag="out_stage", name="out_stage")
                for h in range(2):
                    po = ps_tile("po")
                    for t in range(q_idx + 1):
                        nc.tensor.matmul(
                            po[:, 0:257],
                            lhsT=tiles[h][t][:, bass.ts(m, P)],
                            rhs=v_sb[t][:, h * VOFF:h * VOFF + 257],
                            start=(t == 0), stop=(t == q_idx), skip_group_check=True)
                    rec = outp.tile([P, 1], F32, tag="rec", name="rec")
                    nc.vector.reciprocal(rec[:], po[:, 256:257])
                    nc.scalar.activation(out_stage[:, bass.ts(h, 256)], po[:, 0:256],
                                         mybir.ActivationFunctionType.Copy, scale=rec[:])
                    yield q_idx + 1
                nc.sync.dma_start(out=d_out[bass.ts(q_idx, P), :], in_=out_stage[:])

        # queue of (gate_tile, generator): av(c) must wait for v_sb[4c+3]
        att_queue = []
        for c in range(NCH):
            att_queue.append((-1, gen_scores(c)))
            att_queue.append((4 * c + 3, gen_av(c)))

        def pump(t_done, budget):
            emitted = 0
            while att_queue and emitted < budget:
                gate, gen = att_queue[0]
                if gate > t_done:
                    break
                try:
                    emitted += next(gen)
                except StopIteration:
                    att_queue.pop(0)
            return emitted

        # --- phase 2: V projection with attention work interleaved.
        # Pump in small slices inside each V chain so scores matmuls spread
        # out and the ACT exp drain never backs up the PSUM pool.
        for t in range(NST):
            pv = ps_tile("pv")
            for k in range(NKT):
                nc.tensor.matmul(pv[:], lhsT=xt[k][:, bass.ts(t, P)], rhs=wv_sb[:, k, :],
                                 start=(k == 0), stop=(k == NKT - 1), skip_group_check=True)
                if t > 0 and k in (3, 7, 11):
                    pump(t - 1, 4)
            nc.vector.tensor_copy(v_sb[t][:, 0:256], pv[:, 0:256])
            nc.vector.tensor_copy(v_sb[t][:, VOFF:VOFF + 256], pv[:, 256:512])
            nc.vector.memset(v_sb[t][:, 256:257], 1.0)
            nc.vector.memset(v_sb[t][:, VOFF + 256:VOFF + 257], 1.0)
            pump(t, 12)
        while att_queue:
            pump(NST, 10 ** 9)

    _legalize_waits(nc)
    _dedup_ldweights(nc)
    return nc


def _host_prep(hidden_states, position_ids, Wq, Wk, Wv):
    """Build the 8 per-core input maps."""
    hidden_states = np.asarray(hidden_states, dtype=np.float32)
    position_ids = np.asarray(position_ids)
    Wq = np.asarray(Wq, dtype=np.float32)
    Wk = np.asarray(Wk, dtype=np.float32)
    Wv = np.asarray(Wv, dtype=np.float32)

    scale = 1.0 / np.sqrt(HD)
    tri = np.triu(np.ones((P, P), dtype=np.float32)).astype(ml_dtypes.bfloat16)
    inv_freq = (1.0 / (THETA ** (np.arange(0, HD, 2, dtype=np.float32) / HD))).astype(np.float32)

    in_maps = []
    for c in range(8):
        b, p = c // 4, c % 4
        xt = np.ascontiguousarray(hidden_states[b].T).astype(ml_dtypes.bfloat16)

        def relayout(w):
            # [HID, C] -> dense partition-major [P, NKT*C]
            c = w.shape[1]
            return np.ascontiguousarray(
                w.reshape(NKT, P, c).transpose(1, 0, 2).reshape(P, NKT * c))

        wq = relayout(Wq[:, p * P:(p + 1) * P] * scale).astype(ml_dtypes.bfloat16)
        wk = relayout(Wk[:, p * P:(p + 1) * P]).astype(ml_dtypes.bfloat16)
        cols = []
        for h in (2 * p, 2 * p + 1):
            for r in range(G):
                j = r * HKV + h
                cols.append(Wv[:, j * HD:(j + 1) * HD])
        wv = relayout(np.concatenate(cols, axis=1)).astype(ml_dtypes.bfloat16)

        pos = position_ids[b].astype(np.float32)
        freqs = pos[:, None] * inv_freq[None, :]          # [S, 32]
        cos32 = np.cos(freqs).T.astype(np.float32)        # [32, S]
        sin32 = np.sin(freqs).T.astype(np.float32)
        # [128, S]: rows repeat per 32 (2 heads x [d<32 | d>=32]); sign of the
        # rotate-half product folded into the sin table.
        cost = np.ascontiguousarray(
            np.concatenate([cos32] * 4, axis=0)).astype(ml_dtypes.bfloat16)
        # block-swapped layout: row p holds the sin factor for DST partition
        # swap(p) (32-blocks 0<->1, 2<->3), so rope's t2 reads in0/in1 at the
        # same base partition
        sint = np.ascontiguousarray(
            np.concatenate([sin32, -sin32, sin32, -sin32], axis=0)).astype(ml_dtypes.bfloat16)

        in_maps.append({
            "xt": xt, "wq": wq, "wk": wk, "wv": wv,
            "cost": cost, "sint": sint, "tri": tri,
        })
    return in_maps


def kernel(hidden_states, position_ids, Wq, Wk, Wv):
    global LAST_RESULTS
    trace = bool(os.environ.get("CHEEMS_TRACE"))
    if trace:
        _install_ntff_hook()
    if "nc" not in _CACHE:
        _CACHE["nc"] = _build()
    nc = _CACHE["nc"]
    in_maps = _host_prep(hidden_states, position_ids, Wq, Wk, Wv)
    res = run_bass_kernel_spmd(nc, in_maps, core_ids=list(range(8)), trace=trace)
    LAST_RESULTS = res

    out = np.empty((B, S, HID), dtype=np.float32)
    for c in range(8):
        b, p = c // 4, c % 4
        core_out = res.results[c]["out"]          # [S, 512]
        for hl, h in enumerate((2 * p, 2 * p + 1)):
            for r in range(G):
                j = r * HKV + h
                out[b, :, j * HD:(j + 1) * HD] = core_out[:, (hl * G + r) * HD:(hl * G + r + 1) * HD]
    return out.reshape(B, S, HID)



# revision 20
# speedup vs baseline: 1.2135x; 1.2135x over previous
"""Trainium2 Bass kernel for CheemsNonWoAttn (GQA attention block, no Wo).

Sharding: 8 cores = batch(2) x kv-head-pair(4). Each core handles one batch
element and 2 of the 8 kv heads (GQA: Q and K are repeated identically across
the 4 groups, so only 8 unique softmax matrices exist; V uses all 32 heads).

Per-core device program:
  Qt/Kt = Wq^T X^T, Wk^T X^T    (d on partitions; N=512 matmuls, K accum=16)
  RoPE on DVE (partition-shifted multiplies, sign folded into sin table)
  V = X Wv                      (natural [s, 512] layout)
  scores^T[k, q] = Kt^T Qt      (per head, K=64, row-tiled across array halves)
  E = exp(scores) on ACT        (no max subtraction: |scores| <~ 6, safe)
  causal mask: multiply 128x128 diagonal blocks by triangular mask (DVE)
  out = E^T V with a ones-column in V producing the softmax denominator
  out /= denom (DVE reciprocal + tensor_scalar)
"""
import os
import sys
import types
from contextlib import ExitStack

for _p in ("/opt/trn_rl_repo", "/root/.axon_site/_ro/trn_rl_repo"):
    if os.path.isdir(_p) and _p not in sys.path:
        sys.path.append(_p)

import numpy as np
import ml_dtypes

import concourse.bass as bass
import concourse.tile as tile
from concourse import mybir
from concourse.bass_utils import run_bass_kernel_spmd
from concourse.vector_clock import ScopedClock

# ---------------------------------------------------------------------------
# Patch 1: walrus rejects Drain instructions with >1 sync wait (CTRL ops have
# a single wait slot). Split the TileContext exit drain's waits across extra
# SP nops, one wait each.
def _patched_drain_and_barrier(self, tick_clock, wait_clock):
    nc = self.nc
    drain_bi = nc.sync.drain()
    wait_clock.add_sem_waits(drain_bi.ins, ScopedClock({None: tick_clock.global_clock}))
    inst = drain_bi.ins
    si = inst.sync_info
    if si is not None and si.on_wait is not None and len(si.on_wait) > 1:
        waits = list(si.on_wait)
        inst.sync_info = mybir.SyncInfo(
            on_wait=waits[:1],
            on_update=list(si.on_update) if si.on_update else [],
        )
        for w in waits[1:]:
            nbi = nc.sync.nop()
            nbi.ins.sync_info = mybir.SyncInfo(on_wait=[w], on_update=[])
    nc.all_engine_barrier()
    assert self.sems is not None
    popped = nc._tile_sem_poison_stack.pop()
    assert popped is self._sem_poison
    nc.clear_and_free_semaphores(list(self.sems.allocated().values()))
    nc.all_engine_barrier()


tile.TileContext._drain_and_barrier = _patched_drain_and_barrier


def _legalize_waits(nc):
    """This walrus build accepts at most one sync-wait per instruction.
    Split any instruction carrying N>1 waits into N-1 preceding same-engine
    nops (engines are in-order, so semantics are preserved)."""
    uid = 0
    for f in nc.m.functions:
        for blk in f.blocks:
            insts = list(blk.instructions)
            out, changed = [], False
            for inst in insts:
                si = getattr(inst, "sync_info", None)
                if si is not None and si.on_wait is not None and len(si.on_wait) > 1:
                    waits = list(si.on_wait)
                    for w in waits[:-1]:
                        uid += 1
                        out.append(mybir.InstNoOp(
                            name=f"{inst.name}_lw{uid}",
                            engine=inst.engine,
                            sync_info=mybir.SyncInfo(on_wait=[w], on_update=[]),
                            bass_nofuse=True,
                        ))
                    inst.sync_info = mybir.SyncInfo(
                        on_wait=waits[-1:],
                        on_update=list(si.on_update) if si.on_update else [],
                    )
                    changed = True
                out.append(inst)
            if changed:
                blk.instructions = out


def _dedup_ldweights(nc):
    """Clear the self-load on matmuls whose stationary operand AP is identical
    to the immediately preceding matmul in the final PE stream (walrus runs
    with ldw-opt disabled, so it reloads weights for every matmul otherwise)."""
    if os.environ.get("CHEEMS_NO_LDW_DEDUP"):
        return
    for f in nc.m.functions:
        for blk in f.blocks:
            prev_key = None
            for inst in blk.instructions:
                if not isinstance(inst, mybir.InstMatmult):
                    if isinstance(inst, mybir.InstLdweights):
                        prev_key = None
                    continue
                key = (repr(inst.ins[1]), inst.perf_mode, inst.is_transpose,
                       repr(inst.tile_position))
                if prev_key is not None and key == prev_key:
                    inst.ldweights = False
                prev_key = key


# Patch 2 (optional, for tracing): recreate the antenv.axon_hooks shim so
# run_bass_kernel_spmd(trace=True) can capture NTFF profiles under axon.
def _install_ntff_hook():
    try:
        if "antenv.axon_hooks" in sys.modules:
            return
        import antenv
        from trn_agent_boot.trn_boot import _ntff_profile_via_ctypes

        hook = _ntff_profile_via_ctypes("/opt/axon/libaxon_pjrt.so")
        mod = types.ModuleType("antenv.axon_hooks")
        mod._hook = hook
        mod.get_axon_ntff_profile_hook = lambda: mod._hook

        def _set(h):
            mod._hook = h

        mod.set_axon_ntff_profile_hook = _set
        sys.modules["antenv.axon_hooks"] = mod
        antenv.axon_hooks = mod
    except Exception:
        pass


# ---------------------------------------------------------------------------
B, S, HID = 2, 2048, 2048
NH, G = 32, 4
HD = 64          # head dim
HKV = 8          # kv heads
THETA = 10000.0
P = 128          # partitions
NKT = HID // P   # 16 k-tiles over the contraction dim
NST = S // P     # 16 s-tiles
NCH = 4          # s-chunks of 512
CH = 512
DV = 512         # v columns per core (8 heads x 64)
VROW = 528       # v tile row: [256 v | 1 one | 7 pad] x 2 heads
VOFF = 264

F32 = mybir.dt.float32
BF16 = mybir.dt.bfloat16

_CACHE = {}
LAST_RESULTS = None


def _build():
    nc = bass.Bass("TRN2")
    d_xt = nc.declare_dram_parameter("xt", [HID, S], BF16, isOutput=False)
    # weights pre-transposed on host to partition-major [P, kt*cols] so the
    # DMA is a dense linear stream (strided gathers were ~4x slower)
    d_wq = nc.declare_dram_parameter("wq", [P, NKT * P], BF16, isOutput=False)
    d_wk = nc.declare_dram_parameter("wk", [P, NKT * P], BF16, isOutput=False)
    d_wv = nc.declare_dram_parameter("wv", [P, NKT * DV], BF16, isOutput=False)
    d_cos = nc.declare_dram_parameter("cost", [P, S], BF16, isOutput=False)
    d_sin = nc.declare_dram_parameter("sint", [P, S], BF16, isOutput=False)
    d_tri = nc.declare_dram_parameter("tri", [P, P], BF16, isOutput=False)
    d_out = nc.declare_dram_parameter("out", [S, DV], F32, isOutput=True)

    with tile.TileContext(nc) as tc, ExitStack() as ctx:
        pers = ctx.enter_context(tc.tile_pool(name="pers", bufs=1))
        epool = ctx.enter_context(tc.tile_pool(name="epool", bufs=56))
        work = ctx.enter_context(tc.tile_pool(name="work", bufs=2))
        outp = ctx.enter_context(tc.tile_pool(name="outp", bufs=3))
        psum = ctx.enter_context(tc.tile_pool(name="psum", bufs=8, space="PSUM"))

        def ps_tile(name):
            return psum.tile([P, CH], F32, tag="ps", bufs=8, name=name)

        # --- persistent tiles; DMA order tuned so PE never starves.
        # Enqueue in parallel across 4 engine queues (sync/vector/scalar/
        # gpsimd) so descriptor writes (~1us each) don't serialize the
        # stream start. Per-queue FIFO order still delivers wq/wk first,
        # then the xt tiles (QK proj is PE-paced behind them), then wv
        # (needed when V proj starts right after QK), then rope tables.
        wq_sb = pers.tile([P, NKT, P], BF16, tag="wq")
        wk_sb = pers.tile([P, NKT, P], BF16, tag="wk")
        wv_sb = pers.tile([P, NKT, DV], BF16, tag="wv")
        cos_sb = pers.tile([P, S], BF16, tag="cos")
        sin_sb = pers.tile([P, S], BF16, tag="sin")
        tri_sb = pers.tile([P, P], BF16, tag="tri")
        xt = [pers.tile([P, S], BF16, tag=f"xt{k}", name=f"xt{k}") for k in range(NKT)]
        # xt streams in k-order on the sync HW queue at full bandwidth (QK
        # proj consumes tiles serially); weights/tables ride the scalar HW
        # queue in parallel. gpsimd DMA is software-DGE (slow) - avoid.
        # everything on the sync HW queue in consumption order (the scalar
        # queue was observed to start ~10us late, so parallel queues only
        # hurt); wq/wk first since the first QK matmul needs them.
        nc.sync.dma_start(out=wq_sb[:], in_=d_wq[:].rearrange("p (kt m) -> p kt m", m=P))
        nc.sync.dma_start(out=wk_sb[:], in_=d_wk[:].rearrange("p (kt m) -> p kt m", m=P))
        for k in range(8):
            nc.sync.dma_start(out=xt[k][:], in_=d_xt[bass.ts(k, P), :])
        nc.sync.dma_start(out=cos_sb[:], in_=d_cos[:])
        nc.sync.dma_start(out=sin_sb[:], in_=d_sin[:])
        for k in range(8, NKT):
            nc.sync.dma_start(out=xt[k][:], in_=d_xt[bass.ts(k, P), :])
        wv_r = d_wv[:].rearrange("p (kt n) -> p kt n", n=DV)
        nc.sync.dma_start(out=wv_sb[:, 0:8, :], in_=wv_r[:, 0:8, :])
        nc.sync.dma_start(out=wv_sb[:, 8:NKT, :], in_=wv_r[:, 8:NKT, :])
        nc.scalar.dma_start(out=tri_sb[:], in_=d_tri[:])

        qt = pers.tile([P, S], BF16, tag="qt")
        kt = pers.tile([P, S], BF16, tag="kt")
        v_sb = [pers.tile([P, VROW], BF16, tag=f"v{t}", name=f"v{t}") for t in range(NST)]

        # --- phase 1: Q and K projections interleaved per xt tile (8 MMs per
        # DMA arrival keeps PE duty high while the xt stream lands).
        pq = [ps_tile(f"pq{c}") for c in range(NCH)]
        pk = [ps_tile(f"pk{c}") for c in range(NCH)]
        for k in range(NKT):
            for c in range(NCH):
                nc.tensor.matmul(pq[c][:], lhsT=wq_sb[:, k, :], rhs=xt[k][:, bass.ts(c, CH)],
                                 start=(k == 0), stop=(k == NKT - 1), skip_group_check=True)
            for c in range(NCH):
                nc.tensor.matmul(pk[c][:], lhsT=wk_sb[:, k, :], rhs=xt[k][:, bass.ts(c, CH)],
                                 start=(k == 0), stop=(k == NKT - 1), skip_group_check=True)

        def rope(src_ps, dst, c, eng):
            # stage PSUM->bf16 on ACT, then all-bf16 rope ops (2x DVE rate,
            # and gpsimd-eligible since no PSUM reads)
            cs = bass.ts(c, CH)
            qb = work.tile([P, CH], BF16, tag="qb", name="qb")
            nc.scalar.copy(qb[:], src_ps[:])
            t1 = work.tile([P, CH], BF16, tag="t1", name="t1")
            t2 = work.tile([P, CH], BF16, tag="t2", name="t2")
            eng.tensor_mul(out=t1[:], in0=qb[:], in1=cos_sb[:, cs])
            for blk in range(4):
                lo, hi = blk * 32, (blk + 1) * 32
                swap_lo = (blk ^ 1) * 32
                # sin table is block-swapped on host so both SBUF inputs
                # share a base partition (BIR verifier NCC_IBIR297)
                eng.tensor_mul(out=t2[lo:hi, :], in0=qb[swap_lo:swap_lo + 32, :],
                               in1=sin_sb[swap_lo:swap_lo + 32, cs])
            eng.tensor_add(out=dst[:, cs], in0=t1[:], in1=t2[:])

        # all-bf16 rope on DVE (gpsimd tensor ops measured ~2us each - far
        # too slow); k before q per chunk since scores(c) needs kt first
        for c in range(NCH):
            rope(pk[c], kt, c, nc.vector)
            rope(pq[c], qt, c, nc.vector)

        # --- attention work generators (pumped between V-proj s-tiles) ---
        e_tiles = {}

        def gen_scores(c):
            cs0 = c * CH
            nk = 4 * c + 4
            tiles = [[None] * nk for _ in range(2)]
            e_tiles[c] = tiles
            for t in range(nk):
                m = t - 4 * c
                off = max(m, 0) * P
                w = CH - off
                for h in range(2):
                    ps_s = ps_tile("ps_s")
                    nc.tensor.matmul(
                        ps_s[:, 0:w],
                        lhsT=kt[h * HD:(h + 1) * HD, bass.ts(t, P)],
                        rhs=qt[h * HD:(h + 1) * HD, bass.ds(cs0 + off, w)],
                        start=True, stop=True, skip_group_check=True)
                    e = epool.tile([P, CH], BF16, tag="e", name=f"e{h}_{t}")
                    nc.scalar.activation(e[:, bass.ds(off, w)], ps_s[:, 0:w],
                                         mybir.ActivationFunctionType.Exp)
                    if m >= 0:
                        nc.vector.tensor_mul(out=e[:, bass.ts(m, P)],
                                             in0=e[:, bass.ts(m, P)], in1=tri_sb[:])
                    tiles[h][t] = e
                yield 2  # 2 MMs emitted

        def gen_av(c):
            tiles = e_tiles.pop(c)
            for m in range(4):
                q_idx = 4 * c + m
                out_stage = outp.tile([P, DV], F32, tag="out_stage", name="out_stage")
                for h in range(2):
                    po = ps_tile("po")
                    for t in range(q_idx + 1):
                        nc.tensor.matmul(
                            po[:, 0:257],
                            lhsT=tiles[h][t][:, bass.ts(m, P)],
                            rhs=v_sb[t][:, h * VOFF:h * VOFF + 257],
                            start=(t == 0), stop=(t == q_idx), skip_group_check=True)
                    rec = outp.tile([P, 1], F32, tag="rec", name="rec")
                    nc.vector.reciprocal(rec[:], po[:, 256:257])
                    nc.scalar.activation(out_stage[:, bass.ts(h, 256)], po[:, 0:256],
                                         mybir.ActivationFunctionType.Copy, scale=rec[:])
                    yield q_idx + 1
                nc.sync.dma_start(out=d_out[bass.ts(q_idx, P), :], in_=out_stage[:])

        # queue of (gate_tile, generator): av(c) must wait for v_sb[4c+3]
        att_queue = []
        for c in range(NCH):
            att_queue.append((-1, gen_scores(c)))
            att_queue.append((4 * c + 3, gen_av(c)))

        def pump(t_done, budget):
            emitted = 0
            while att_queue and emitted < budget:
                gate, gen = att_queue[0]
                if gate > t_done:
                    break
                try:
                    emitted += next(gen)
                except StopIteration:
                    att_queue.pop(0)
            return emitted

        # --- phase 2: V projection with attention work interleaved.
        # Pump in small slices inside each V chain so scores matmuls spread
        # out and the ACT exp drain never backs up the PSUM pool.
        for t in range(NST):
            pv = ps_tile("pv")
            for k in range(NKT):
                nc.tensor.matmul(pv[:], lhsT=xt[k][:, bass.ts(t, P)], rhs=wv_sb[:, k, :],
                                 start=(k == 0), stop=(k == NKT - 1), skip_group_check=True)
                if t > 0 and k in (3, 7, 11):
                    pump(t - 1, 4)
            nc.vector.tensor_copy(v_sb[t][:, 0:256], pv[:, 0:256])
            nc.vector.tensor_copy(v_sb[t][:, VOFF:VOFF + 256], pv[:, 256:512])
            nc.vector.memset(v_sb[t][:, 256:257], 1.0)
            nc.vector.memset(v_sb[t][:, VOFF + 256:VOFF + 257], 1.0)
            pump(t, 12)
        while att_queue:
            pump(NST, 10 ** 9)

    _legalize_waits(nc)
    _dedup_ldweights(nc)
    return nc


def _host_prep(hidden_states, position_ids, Wq, Wk, Wv):
    """Build the 8 per-core input maps."""
    hidden_states = np.asarray(hidden_states, dtype=np.float32)
    position_ids = np.asarray(position_ids)
    Wq = np.asarray(Wq, dtype=np.float32)
    Wk = np.asarray(Wk, dtype=np.float32)
    Wv = np.asarray(Wv, dtype=np.float32)

    scale = 1.0 / np.sqrt(HD)
    tri = np.triu(np.ones((P, P), dtype=np.float32)).astype(ml_dtypes.bfloat16)
    inv_freq = (1.0 / (THETA ** (np.arange(0, HD, 2, dtype=np.float32) / HD))).astype(np.float32)

    in_maps = []
    for c in range(8):
        b, p = c // 4, c % 4
        xt = np.ascontiguousarray(hidden_states[b].T).astype(ml_dtypes.bfloat16)

        def relayout(w):
            # [HID, C] -> dense partition-major [P, NKT*C]
            c = w.shape[1]
            return np.ascontiguousarray(
                w.reshape(NKT, P, c).transpose(1, 0, 2).reshape(P, NKT * c))

        wq = relayout(Wq[:, p * P:(p + 1) * P] * scale).astype(ml_dtypes.bfloat16)
        wk = relayout(Wk[:, p * P:(p + 1) * P]).astype(ml_dtypes.bfloat16)
        cols = []
        for h in (2 * p, 2 * p + 1):
            for r in range(G):
                j = r * HKV + h
                cols.append(Wv[:, j * HD:(j + 1) * HD])
        wv = relayout(np.concatenate(cols, axis=1)).astype(ml_dtypes.bfloat16)

        pos = position_ids[b].astype(np.float32)
        freqs = pos[:, None] * inv_freq[None, :]          # [S, 32]
        cos32 = np.cos(freqs).T.astype(np.float32)        # [32, S]
        sin32 = np.sin(freqs).T.astype(np.float32)
        # [128, S]: rows repeat per 32 (2 heads x [d<32 | d>=32]); sign of the
        # rotate-half product folded into the sin table.
        cost = np.ascontiguousarray(
            np.concatenate([cos32] * 4, axis=0)).astype(ml_dtypes.bfloat16)
        # block-swapped layout: row p holds the sin factor for DST partition
        # swap(p) (32-blocks 0<->1, 2<->3), so rope's t2 reads in0/in1 at the
        # same base partition
        sint = np.ascontiguousarray(
            np.concatenate([sin32, -sin32, sin32, -sin32], axis=0)).astype(ml_dtypes.bfloat16)

        in_maps.append({
            "xt": xt, "wq": wq, "wk": wk, "wv": wv,
            "cost": cost, "sint": sint, "tri": tri,
        })
    return in_maps


def kernel(hidden_states, position_ids, Wq, Wk, Wv):
    global LAST_RESULTS
    trace = bool(os.environ.get("CHEEMS_TRACE"))
    if trace:
        _install_ntff_hook()
    if "nc" not in _CACHE:
        _CACHE["nc"] = _build()
    nc = _CACHE["nc"]
    in_maps = _host_prep(hidden_states, position_ids, Wq, Wk, Wv)
    res = run_bass_kernel_spmd(nc, in_maps, core_ids=list(range(8)), trace=trace)
    LAST_RESULTS = res

    out = np.empty((B, S, HID), dtype=np.float32)
    for c in range(8):
        b, p = c // 4, c % 4
        core_out = res.results[c]["out"]          # [S, 512]
        for hl, h in enumerate((2 * p, 2 * p + 1)):
            for r in range(G):
                j = r * HKV + h
                out[b, :, j * HD:(j + 1) * HD] = core_out[:, (hl * G + r) * HD:(hl * G + r + 1) * HD]
    return out.reshape(B, S, HID)



# revision 21
# speedup vs baseline: 1.2152x; 1.0014x over previous
"""Trainium2 Bass kernel for CheemsNonWoAttn (GQA attention block, no Wo).

Sharding: 8 cores = batch(2) x kv-head-pair(4). Each core handles one batch
element and 2 of the 8 kv heads (GQA: Q and K are repeated identically across
the 4 groups, so only 8 unique softmax matrices exist; V uses all 32 heads).

Per-core device program:
  Qt/Kt = Wq^T X^T, Wk^T X^T    (d on partitions; N=512 matmuls, K accum=16)
  RoPE on DVE (partition-shifted multiplies, sign folded into sin table)
  V = X Wv                      (natural [s, 512] layout)
  scores^T[k, q] = Kt^T Qt      (per head, K=64, row-tiled across array halves)
  E = exp(scores) on ACT        (no max subtraction: |scores| <~ 6, safe)
  causal mask: multiply 128x128 diagonal blocks by triangular mask (DVE)
  out = E^T V with a ones-column in V producing the softmax denominator
  out /= denom (DVE reciprocal + tensor_scalar)
"""
import os
import sys
import types
from contextlib import ExitStack

for _p in ("/opt/trn_rl_repo", "/root/.axon_site/_ro/trn_rl_repo"):
    if os.path.isdir(_p) and _p not in sys.path:
        sys.path.append(_p)

import numpy as np
import ml_dtypes

import concourse.bass as bass
import concourse.tile as tile
from concourse import mybir
from concourse.bass_utils import run_bass_kernel_spmd
from concourse.vector_clock import ScopedClock

# ---------------------------------------------------------------------------
# Patch 1: walrus rejects Drain instructions with >1 sync wait (CTRL ops have
# a single wait slot). Split the TileContext exit drain's waits across extra
# SP nops, one wait each.
def _patched_drain_and_barrier(self, tick_clock, wait_clock):
    nc = self.nc
    drain_bi = nc.sync.drain()
    wait_clock.add_sem_waits(drain_bi.ins, ScopedClock({None: tick_clock.global_clock}))
    inst = drain_bi.ins
    si = inst.sync_info
    if si is not None and si.on_wait is not None and len(si.on_wait) > 1:
        waits = list(si.on_wait)
        inst.sync_info = mybir.SyncInfo(
            on_wait=waits[:1],
            on_update=list(si.on_update) if si.on_update else [],
        )
        for w in waits[1:]:
            nbi = nc.sync.nop()
            nbi.ins.sync_info = mybir.SyncInfo(on_wait=[w], on_update=[])
    nc.all_engine_barrier()
    assert self.sems is not None
    popped = nc._tile_sem_poison_stack.pop()
    assert popped is self._sem_poison
    nc.clear_and_free_semaphores(list(self.sems.allocated().values()))
    nc.all_engine_barrier()


tile.TileContext._drain_and_barrier = _patched_drain_and_barrier


def _legalize_waits(nc):
    """This walrus build accepts at most one sync-wait per instruction.
    Split any instruction carrying N>1 waits into N-1 preceding same-engine
    nops (engines are in-order, so semantics are preserved)."""
    uid = 0
    for f in nc.m.functions:
        for blk in f.blocks:
            insts = list(blk.instructions)
            out, changed = [], False
            for inst in insts:
                si = getattr(inst, "sync_info", None)
                if si is not None and si.on_wait is not None and len(si.on_wait) > 1:
                    waits = list(si.on_wait)
                    for w in waits[:-1]:
                        uid += 1
                        out.append(mybir.InstNoOp(
                            name=f"{inst.name}_lw{uid}",
                            engine=inst.engine,
                            sync_info=mybir.SyncInfo(on_wait=[w], on_update=[]),
                            bass_nofuse=True,
                        ))
                    inst.sync_info = mybir.SyncInfo(
                        on_wait=waits[-1:],
                        on_update=list(si.on_update) if si.on_update else [],
                    )
                    changed = True
                out.append(inst)
            if changed:
                blk.instructions = out


def _dedup_ldweights(nc):
    """Clear the self-load on matmuls whose stationary operand AP is identical
    to the immediately preceding matmul in the final PE stream (walrus runs
    with ldw-opt disabled, so it reloads weights for every matmul otherwise)."""
    if os.environ.get("CHEEMS_NO_LDW_DEDUP"):
        return
    for f in nc.m.functions:
        for blk in f.blocks:
            prev_key = None
            for inst in blk.instructions:
                if not isinstance(inst, mybir.InstMatmult):
                    if isinstance(inst, mybir.InstLdweights):
                        prev_key = None
                    continue
                key = (repr(inst.ins[1]), inst.perf_mode, inst.is_transpose,
                       repr(inst.tile_position))
                if prev_key is not None and key == prev_key:
                    inst.ldweights = False
                prev_key = key


# Patch 2 (optional, for tracing): recreate the antenv.axon_hooks shim so
# run_bass_kernel_spmd(trace=True) can capture NTFF profiles under axon.
def _install_ntff_hook():
    try:
        if "antenv.axon_hooks" in sys.modules:
            return
        import antenv
        from trn_agent_boot.trn_boot import _ntff_profile_via_ctypes

        hook = _ntff_profile_via_ctypes("/opt/axon/libaxon_pjrt.so")
        mod = types.ModuleType("antenv.axon_hooks")
        mod._hook = hook
        mod.get_axon_ntff_profile_hook = lambda: mod._hook

        def _set(h):
            mod._hook = h

        mod.set_axon_ntff_profile_hook = _set
        sys.modules["antenv.axon_hooks"] = mod
        antenv.axon_hooks = mod
    except Exception:
        pass


# ---------------------------------------------------------------------------
B, S, HID = 2, 2048, 2048
NH, G = 32, 4
HD = 64          # head dim
HKV = 8          # kv heads
THETA = 10000.0
P = 128          # partitions
NKT = HID // P   # 16 k-tiles over the contraction dim
NST = S // P     # 16 s-tiles
NCH = 4          # s-chunks of 512
CH = 512
DV = 512         # v columns per core (8 heads x 64)
VROW = 528       # v tile row: [256 v | 1 one | 7 pad] x 2 heads
VOFF = 264

F32 = mybir.dt.float32
BF16 = mybir.dt.bfloat16

_CACHE = {}
LAST_RESULTS = None


def _build():
    nc = bass.Bass("TRN2")
    d_xt = nc.declare_dram_parameter("xt", [HID, S], BF16, isOutput=False)
    # weights pre-transposed on host to partition-major [P, kt*cols] so the
    # DMA is a dense linear stream (strided gathers were ~4x slower)
    d_wq = nc.declare_dram_parameter("wq", [P, NKT * P], BF16, isOutput=False)
    d_wk = nc.declare_dram_parameter("wk", [P, NKT * P], BF16, isOutput=False)
    d_wv = nc.declare_dram_parameter("wv", [P, NKT * DV], BF16, isOutput=False)
    d_cos = nc.declare_dram_parameter("cost", [P, S], BF16, isOutput=False)
    d_sin = nc.declare_dram_parameter("sint", [P, S], BF16, isOutput=False)
    d_tri = nc.declare_dram_parameter("tri", [P, P], BF16, isOutput=False)
    d_out = nc.declare_dram_parameter("out", [S, DV], F32, isOutput=True)

    with tile.TileContext(nc) as tc, ExitStack() as ctx:
        pers = ctx.enter_context(tc.tile_pool(name="pers", bufs=1))
        epool = ctx.enter_context(tc.tile_pool(name="epool", bufs=56))
        work = ctx.enter_context(tc.tile_pool(name="work", bufs=2))
        outp = ctx.enter_context(tc.tile_pool(name="outp", bufs=3))
        psum = ctx.enter_context(tc.tile_pool(name="psum", bufs=8, space="PSUM"))

        def ps_tile(name):
            return psum.tile([P, CH], F32, tag="ps", bufs=8, name=name)

        # --- persistent tiles; DMA order tuned so PE never starves.
        # Enqueue in parallel across 4 engine queues (sync/vector/scalar/
        # gpsimd) so descriptor writes (~1us each) don't serialize the
        # stream start. Per-queue FIFO order still delivers wq/wk first,
        # then the xt tiles (QK proj is PE-paced behind them), then wv
        # (needed when V proj starts right after QK), then rope tables.
        wq_sb = pers.tile([P, NKT, P], BF16, tag="wq")
        wk_sb = pers.tile([P, NKT, P], BF16, tag="wk")
        wv_sb = pers.tile([P, NKT, DV], BF16, tag="wv")
        cos_sb = pers.tile([P, S], BF16, tag="cos")
        sin_sb = pers.tile([P, S], BF16, tag="sin")
        tri_sb = pers.tile([P, P], BF16, tag="tri")
        xt = [pers.tile([P, S], BF16, tag=f"xt{k}", name=f"xt{k}") for k in range(NKT)]
        # xt streams in k-order on the sync HW queue at full bandwidth (QK
        # proj consumes tiles serially); weights/tables ride the scalar HW
        # queue in parallel. gpsimd DMA is software-DGE (slow) - avoid.
        # everything on the sync HW queue in consumption order (the scalar
        # queue was observed to start ~10us late, so parallel queues only
        # hurt); wq/wk first since the first QK matmul needs them.
        nc.sync.dma_start(out=wq_sb[:], in_=d_wq[:].rearrange("p (kt m) -> p kt m", m=P))
        nc.sync.dma_start(out=wk_sb[:], in_=d_wk[:].rearrange("p (kt m) -> p kt m", m=P))
        for k in range(8):
            nc.sync.dma_start(out=xt[k][:], in_=d_xt[bass.ts(k, P), :])
        nc.sync.dma_start(out=cos_sb[:], in_=d_cos[:])
        nc.sync.dma_start(out=sin_sb[:], in_=d_sin[:])
        for k in range(8, NKT):
            nc.sync.dma_start(out=xt[k][:], in_=d_xt[bass.ts(k, P), :])
        wv_r = d_wv[:].rearrange("p (kt n) -> p kt n", n=DV)
        nc.sync.dma_start(out=wv_sb[:, 0:8, :], in_=wv_r[:, 0:8, :])
        nc.sync.dma_start(out=wv_sb[:, 8:NKT, :], in_=wv_r[:, 8:NKT, :])
        nc.scalar.dma_start(out=tri_sb[:], in_=d_tri[:])

        qt = pers.tile([P, S], BF16, tag="qt")
        kt = pers.tile([P, S], BF16, tag="kt")
        v_sb = [pers.tile([P, VROW], BF16, tag=f"v{t}", name=f"v{t}") for t in range(NST)]

        # --- phase 1: Q and K projections interleaved per xt tile (8 MMs per
        # DMA arrival keeps PE duty high while the xt stream lands).
        pq = [ps_tile(f"pq{c}") for c in range(NCH)]
        pk = [ps_tile(f"pk{c}") for c in range(NCH)]
        for k in range(NKT):
            for c in range(NCH):
                nc.tensor.matmul(pq[c][:], lhsT=wq_sb[:, k, :], rhs=xt[k][:, bass.ts(c, CH)],
                                 start=(k == 0), stop=(k == NKT - 1), skip_group_check=True)
            for c in range(NCH):
                nc.tensor.matmul(pk[c][:], lhsT=wk_sb[:, k, :], rhs=xt[k][:, bass.ts(c, CH)],
                                 start=(k == 0), stop=(k == NKT - 1), skip_group_check=True)

        def rope(src_ps, dst, c, eng):
            # stage PSUM->bf16 on ACT, then all-bf16 rope ops (2x DVE rate,
            # and gpsimd-eligible since no PSUM reads)
            cs = bass.ts(c, CH)
            qb = work.tile([P, CH], BF16, tag="qb", name="qb")
            nc.scalar.copy(qb[:], src_ps[:])
            t1 = work.tile([P, CH], BF16, tag="t1", name="t1")
            t2 = work.tile([P, CH], BF16, tag="t2", name="t2")
            eng.tensor_mul(out=t1[:], in0=qb[:], in1=cos_sb[:, cs])
            for blk in range(4):
                lo, hi = blk * 32, (blk + 1) * 32
                swap_lo = (blk ^ 1) * 32
                # sin table is block-swapped on host so both SBUF inputs
                # share a base partition (BIR verifier NCC_IBIR297)
                eng.tensor_mul(out=t2[lo:hi, :], in0=qb[swap_lo:swap_lo + 32, :],
                               in1=sin_sb[swap_lo:swap_lo + 32, cs])
            eng.tensor_add(out=dst[:, cs], in0=t1[:], in1=t2[:])

        # all-bf16 rope on DVE (gpsimd tensor ops measured ~2us each - far
        # too slow); k before q per chunk since scores(c) needs kt first
        for c in range(NCH):
            rope(pk[c], kt, c, nc.vector)
            rope(pq[c], qt, c, nc.vector)

        # --- attention work generators (pumped between V-proj s-tiles) ---
        e_tiles = {}

        def gen_scores(c):
            cs0 = c * CH
            nk = 4 * c + 4
            tiles = [[None] * nk for _ in range(2)]
            e_tiles[c] = tiles
            # head-major so consecutive scores matmuls keep the same PE row
            # group (h0 rows 0-63 / h1 rows 64-127); switching each matmul
            # cost ~200ns extra
            for h in range(2):
                for t in range(nk):
                    m = t - 4 * c
                    off = max(m, 0) * P
                    w = CH - off
                    ps_s = ps_tile("ps_s")
                    nc.tensor.matmul(
                        ps_s[:, 0:w],
                        lhsT=kt[h * HD:(h + 1) * HD, bass.ts(t, P)],
                        rhs=qt[h * HD:(h + 1) * HD, bass.ds(cs0 + off, w)],
                        start=True, stop=True, skip_group_check=True)
                    e = epool.tile([P, CH], BF16, tag="e", name=f"e{h}_{t}")
                    nc.scalar.activation(e[:, bass.ds(off, w)], ps_s[:, 0:w],
                                         mybir.ActivationFunctionType.Exp)
                    if m >= 0:
                        nc.vector.tensor_mul(out=e[:, bass.ts(m, P)],
                                             in0=e[:, bass.ts(m, P)], in1=tri_sb[:])
                    tiles[h][t] = e
                    yield 1

        def gen_av(c):
            tiles = e_tiles.pop(c)
            for m in range(4):
                q_idx = 4 * c + m
                out_stage = outp.tile([P, DV], F32, tag="out_stage", name="out_stage")
                for h in range(2):
                    po = ps_tile("po")
                    for t in range(q_idx + 1):
                        nc.tensor.matmul(
                            po[:, 0:257],
                            lhsT=tiles[h][t][:, bass.ts(m, P)],
                            rhs=v_sb[t][:, h * VOFF:h * VOFF + 257],
                            start=(t == 0), stop=(t == q_idx), skip_group_check=True)
                    rec = outp.tile([P, 1], F32, tag="rec", name="rec")
                    nc.vector.reciprocal(rec[:], po[:, 256:257])
                    nc.scalar.activation(out_stage[:, bass.ts(h, 256)], po[:, 0:256],
                                         mybir.ActivationFunctionType.Copy, scale=rec[:])
                    yield q_idx + 1
                nc.sync.dma_start(out=d_out[bass.ts(q_idx, P), :], in_=out_stage[:])

        # queue of (gate_tile, generator): av(c) must wait for v_sb[4c+3]
        att_queue = []
        for c in range(NCH):
            att_queue.append((-1, gen_scores(c)))
            att_queue.append((4 * c + 3, gen_av(c)))

        def pump(t_done, budget):
            emitted = 0
            while att_queue and emitted < budget:
                gate, gen = att_queue[0]
                if gate > t_done:
                    break
                try:
                    emitted += next(gen)
                except StopIteration:
                    att_queue.pop(0)
            return emitted

        # --- phase 2: V projection with attention work interleaved.
        # Pump in small slices inside each V chain so scores matmuls spread
        # out and the ACT exp drain never backs up the PSUM pool.
        for t in range(NST):
            pv = ps_tile("pv")
            for k in range(NKT):
                nc.tensor.matmul(pv[:], lhsT=xt[k][:, bass.ts(t, P)], rhs=wv_sb[:, k, :],
                                 start=(k == 0), stop=(k == NKT - 1), skip_group_check=True)
                if t > 0 and k in (3, 7, 11):
                    pump(t - 1, 4)
            nc.vector.tensor_copy(v_sb[t][:, 0:256], pv[:, 0:256])
            nc.vector.tensor_copy(v_sb[t][:, VOFF:VOFF + 256], pv[:, 256:512])
            nc.vector.memset(v_sb[t][:, 256:257], 1.0)
            nc.vector.memset(v_sb[t][:, VOFF + 256:VOFF + 257], 1.0)
            pump(t, 12)
        while att_queue:
            pump(NST, 10 ** 9)

    _legalize_waits(nc)
    _dedup_ldweights(nc)
    return nc


def _host_prep(hidden_states, position_ids, Wq, Wk, Wv):
    """Build the 8 per-core input maps."""
    hidden_states = np.asarray(hidden_states, dtype=np.float32)
    position_ids = np.asarray(position_ids)
    Wq = np.asarray(Wq, dtype=np.float32)
    Wk = np.asarray(Wk, dtype=np.float32)
    Wv = np.asarray(Wv, dtype=np.float32)

    scale = 1.0 / np.sqrt(HD)
    tri = np.triu(np.ones((P, P), dtype=np.float32)).astype(ml_dtypes.bfloat16)
    inv_freq = (1.0 / (THETA ** (np.arange(0, HD, 2, dtype=np.float32) / HD))).astype(np.float32)

    in_maps = []
    for c in range(8):
        b, p = c // 4, c % 4
        xt = np.ascontiguousarray(hidden_states[b].T).astype(ml_dtypes.bfloat16)

        def relayout(w):
            # [HID, C] -> dense partition-major [P, NKT*C]
            c = w.shape[1]
            return np.ascontiguousarray(
                w.reshape(NKT, P, c).transpose(1, 0, 2).reshape(P, NKT * c))

        wq = relayout(Wq[:, p * P:(p + 1) * P] * scale).astype(ml_dtypes.bfloat16)
        wk = relayout(Wk[:, p * P:(p + 1) * P]).astype(ml_dtypes.bfloat16)
        cols = []
        for h in (2 * p, 2 * p + 1):
            for r in range(G):
                j = r * HKV + h
                cols.append(Wv[:, j * HD:(j + 1) * HD])
        wv = relayout(np.concatenate(cols, axis=1)).astype(ml_dtypes.bfloat16)

        pos = position_ids[b].astype(np.float32)
        freqs = pos[:, None] * inv_freq[None, :]          # [S, 32]
        cos32 = np.cos(freqs).T.astype(np.float32)        # [32, S]
        sin32 = np.sin(freqs).T.astype(np.float32)
        # [128, S]: rows repeat per 32 (2 heads x [d<32 | d>=32]); sign of the
        # rotate-half product folded into the sin table.
        cost = np.ascontiguousarray(
            np.concatenate([cos32] * 4, axis=0)).astype(ml_dtypes.bfloat16)
        # block-swapped layout: row p holds the sin factor for DST partition
        # swap(p) (32-blocks 0<->1, 2<->3), so rope's t2 reads in0/in1 at the
        # same base partition
        sint = np.ascontiguousarray(
            np.concatenate([sin32, -sin32, sin32, -sin32], axis=0)).astype(ml_dtypes.bfloat16)

        in_maps.append({
            "xt": xt, "wq": wq, "wk": wk, "wv": wv,
            "cost": cost, "sint": sint, "tri": tri,
        })
    return in_maps


def kernel(hidden_states, position_ids, Wq, Wk, Wv):
    global LAST_RESULTS
    trace = bool(os.environ.get("CHEEMS_TRACE"))
    if trace:
        _install_ntff_hook()
    if "nc" not in _CACHE:
        _CACHE["nc"] = _build()
    nc = _CACHE["nc"]
    in_maps = _host_prep(hidden_states, position_ids, Wq, Wk, Wv)
    res = run_bass_kernel_spmd(nc, in_maps, core_ids=list(range(8)), trace=trace)
    LAST_RESULTS = res

    out = np.empty((B, S, HID), dtype=np.float32)
    for c in range(8):
        b, p = c // 4, c % 4
        core_out = res.results[c]["out"]          # [S, 512]
        for hl, h in enumerate((2 * p, 2 * p + 1)):
            for r in range(G):
                j = r * HKV + h
                out[b, :, j * HD:(j + 1) * HD] = core_out[:, (hl * G + r) * HD:(hl * G + r + 1) * HD]
    return out.reshape(B, S, HID)

